# revision 6
# baseline (speedup 1.0000x reference)
"""Trainium2 Bass kernel for nn_BranchingLayer (gnn_message_passing).

Computation (per event e of 131072):
    h    = leaky_relu(concat(x[idx[e]], g[idx[e] % E]) @ W1 + b1)   # 192 -> 512
    proj = h @ W2 + b2                                              # 512 -> 256
    out  = concat([x, children]) where children[b*E + e] = proj[e, b*128:(b+1)*128]

Strategy: pure data parallel over events across 8 NeuronCores. The whole MLP
runs in "transposed" space (features on SBUF partitions, events on the free
dim): every DMA is contiguous, b1/b2 become per-partition biases, and no
on-device transposes are needed. Host does the (identity) gather, the layout
transposes, the x passthrough and the final concat.

Matmuls run in bf16 (1 PE cycle/row; fp32 is 4 cycles/row) with fp32 PSUM
accumulation. LeakyReLU+b1 is fused into one ScalarE activation per chunk
(Lrelu, alpha=0.01); b2 is a per-partition DVE tensor_scalar add fused with
the PSUM eviction.
"""

import os
import numpy as np

E_TOT = 131072
N_CORES = 8
E = E_TOT // N_CORES  # 16384 events per core
F = 128               # node features
G = 64                # global features
H = 512               # hidden
EV = 512              # event-block width (one PSUM bank at fp32)
NBLK = E // EV        # 32 blocks per core

_NC = None            # compiled bass module (built once per process)
LAST_RESULTS = None   # BassKernelResults of the most recent run (for test.py)


def _build_nc():
    from contextlib import ExitStack

    import concourse.tile as tile
    from concourse import bacc, mybir

    f32 = mybir.dt.float32
    bf16 = mybir.dt.bfloat16
    LR = mybir.ActivationFunctionType.Lrelu

    nc = bacc.Bacc("TRN2", target_bir_lowering=False)

    xt_d = nc.dram_tensor("xt", [F, E], bf16, kind="ExternalInput")
    gt_d = nc.dram_tensor("gt", [G, E], bf16, kind="ExternalInput")
    w1a_d = nc.dram_tensor("w1a", [128, H], bf16, kind="ExternalInput")
    w1b_d = nc.dram_tensor("w1b", [G, H], bf16, kind="ExternalInput")
    # w2c[p, h*256+n] = W2[h*128+p, n]
    w2c_d = nc.dram_tensor("w2c", [128, 4 * 256], bf16, kind="ExternalInput")
    b1c_d = nc.dram_tensor("b1c", [128, 4], f32, kind="ExternalInput")
    b2c_d = nc.dram_tensor("b2c", [128, 2], f32, kind="ExternalInput")
    out_d = nc.dram_tensor("outT", [256, E], bf16, kind="ExternalOutput")

    with ExitStack() as ctx:
        tc = ctx.enter_context(tile.TileContext(nc))
        const = ctx.enter_context(tc.tile_pool(name="const", bufs=1))
        xpool = ctx.enter_context(tc.tile_pool(name="xp", bufs=3))
        gpool = ctx.enter_context(tc.tile_pool(name="gp", bufs=3))
        htps = ctx.enter_context(tc.tile_pool(name="htps", bufs=4, space="PSUM"))
        htsb = ctx.enter_context(tc.tile_pool(name="htsb", bufs=8))
        pjps = ctx.enter_context(tc.tile_pool(name="pjps", bufs=4, space="PSUM"))
        outp = ctx.enter_context(tc.tile_pool(name="outp", bufs=4))

        w1a_sb = const.tile_from(w1a_d[:])
        w1b_sb = const.tile_from(w1b_d[:])
        w2c_sb = const.tile_from(w2c_d[:])
        b1c_sb = const.tile_from(b1c_d[:])
        b2c_sb = const.tile_from(b2c_d[:])

        for blk in range(NBLK):
            e0 = blk * EV
            xt = xpool.tile([F, EV], bf16)
            nc.sync.dma_start(xt[:], xt_d[:, e0 : e0 + EV])
            gt = gpool.tile([G, EV], bf16)
            nc.sync.dma_start(gt[:], gt_d[:, e0 : e0 + EV])

            hts = []
            for h in range(4):
                htp = htps.tile([128, EV], f32)
                nc.tensor.matmul(
                    htp[:],
                    w1a_sb[:, h * 128 : (h + 1) * 128],
                    xt[:],
                    start=True,
                    stop=False,
                )
                nc.tensor.matmul(
                    htp[:],
                    w1b_sb[:, h * 128 : (h + 1) * 128],
                    gt[:],
                    start=False,
                    stop=True,
                )
                hsb = htsb.tile([128, EV], bf16)
                nc.scalar.activation(
                    hsb[:], htp[:], LR, bias=b1c_sb[:, h : h + 1], scale=1.0,
                    alpha=0.01,
                )
                hts.append(hsb)

            for j in range(2):
                pjp = pjps.tile([128, EV], f32)
                for h in range(4):
                    nc.tensor.matmul(
                        pjp[:],
                        w2c_sb[:, h * 256 + j * 128 : h * 256 + (j + 1) * 128],
                        hts[h][:],
                        start=(h == 0),
                        stop=(h == 3),
                    )
                osb = outp.tile([128, EV], bf16)
                nc.vector.tensor_scalar_add(osb[:], pjp[:], b2c_sb[:, j : j + 1])
                nc.sync.dma_start(out_d[j * 128 : (j + 1) * 128, e0 : e0 + EV], osb[:])

    nc.compile()
    return nc


def _get_nc():
    global _NC
    if _NC is None:
        _NC = _build_nc()
    return _NC


def kernel(**inputs):
    global LAST_RESULTS
    import ml_dtypes
    from concourse.bass_utils import run_bass_kernel_spmd

    bf16 = ml_dtypes.bfloat16

    x = np.ascontiguousarray(np.asarray(inputs["x"], dtype=np.float32))
    g = np.ascontiguousarray(np.asarray(inputs["global_features"], dtype=np.float32))
    W1 = np.asarray(inputs["W1"], dtype=np.float32)
    b1 = np.asarray(inputs["b1"], dtype=np.float32)
    W2 = np.asarray(inputs["W2"], dtype=np.float32)
    b2 = np.asarray(inputs["b2"], dtype=np.float32)
    idx = np.asarray(inputs["parents_idxs"])

    n_events = x.shape[0]
    assert n_events == E_TOT and idx.shape[0] == E_TOT

    # Gather (identity when parents_idxs == arange, the spec'd fill).
    if idx[0] == 0 and idx[-1] == n_events - 1 and np.array_equal(
        idx, np.arange(n_events, dtype=idx.dtype)
    ):
        xg, gg = x, g
    else:
        xg = x[idx]
        gg = g[idx % n_events]

    xT = np.ascontiguousarray(xg.T.astype(bf16))  # [F, E_TOT]
    gT = np.ascontiguousarray(gg.T.astype(bf16))  # [G, E_TOT]
    w1a = np.ascontiguousarray(W1[:128, :].astype(bf16))
    w1b = np.ascontiguousarray(W1[128:192, :].astype(bf16))
    w2c = np.ascontiguousarray(
        W2.reshape(4, 128, 256).transpose(1, 0, 2).reshape(128, 1024).astype(bf16)
    )
    b1c = np.ascontiguousarray(b1.reshape(4, 128).T)
    b2c = np.ascontiguousarray(b2.reshape(2, 128).T)

    in_maps = []
    for c in range(N_CORES):
        sl = slice(c * E, (c + 1) * E)
        in_maps.append(
            {
                "xt": np.ascontiguousarray(xT[:, sl]),
                "gt": np.ascontiguousarray(gT[:, sl]),
                "w1a": w1a,
                "w1b": w1b,
                "w2c": w2c,
                "b1c": b1c,
                "b2c": b2c,
            }
        )

    nc = _get_nc()
    LAST_RESULTS = run_bass_kernel_spmd(
        nc, in_maps, core_ids=list(range(N_CORES)),
        trace=bool(int(os.environ.get("KERNEL_TRACE", "0"))),
    )

    out = np.empty((3 * E_TOT, F), dtype=np.float32)
    out[:E_TOT] = x
    for c in range(N_CORES):
        oT = LAST_RESULTS.results[c]["outT"]
        out[E_TOT + c * E : E_TOT + (c + 1) * E] = oT[:128].astype(np.float32).T
        out[2 * E_TOT + c * E : 2 * E_TOT + (c + 1) * E] = oT[128:].astype(np.float32).T
    return out


# revision 30
# speedup vs baseline: 336.4309x; 336.4309x over previous
"""Trainium2 Bass kernel for nn_BranchingLayer (gnn_message_passing).

Computation (per event e of 131072):
    h    = leaky_relu(concat(x[idx[e]], g[idx[e] % E]) @ W1 + b1)   # 192 -> 512
    proj = h @ W2 + b2                                              # 512 -> 256
    out  = concat([x, children]) where children[b*E + e] = proj[e, b*128:(b+1)*128]

Strategy: pure data parallel over events across 8 NeuronCores. The whole MLP
runs in "transposed" space (features on SBUF partitions, events on the free
dim): every DMA is contiguous, b1/b2 become per-partition biases, and no
on-device transposes are needed. Host does the (identity) gather, the layout
transposes, the x passthrough and the final concat.

Device kernel details:
  - bf16 matmuls (1 PE cycle/row vs 4 for fp32), fp32 PSUM accumulation,
    single-bank [128, 512] PSUM tiles.
  - The K=64 global-feature matmuls are row-packed: g/W1b are duplicated on
    partitions 64..127 so two chunks' K=64 matmuls run concurrently in
    disjoint PE row groups.
  - LeakyReLU+b1 fused into one ScalarE Lrelu per hidden chunk; b2 fused into
    the DVE PSUM eviction as a per-partition tensor_scalar add.
  - 2048-event input DMA chunks and 2048-event output staging buffers.
"""

import os
import numpy as np

E_TOT = 131072
N_CORES = 8
E = E_TOT // N_CORES  # 16384 events per core
F = 128               # node features
G = 64                # global features
H = 512               # hidden
EV = 512              # compute block width (one PSUM bank at fp32)
LV = 2048             # input-DMA / output-staging chunk width (events)
NCH = E // LV         # 8 chunks per core

_NC = None            # compiled bass module (built once per process)
LAST_RESULTS = None   # BassKernelResults of the most recent run (for test.py)
LAST_IN_MAPS = None   # per-core input dicts of the most recent run (for test.py)


def _build_nc(reps=1, pack=True, use_loop=False, store_gpsimd=False):
    """Build the bass module. reps>1 repeats the whole body (identical work
    each iteration; For_i when use_loop else python-unroll) — used only for
    timing benchmarks. pack=False disables the row-packed K=64 matmuls.
    store_gpsimd routes output stores through SWDGE so they don't contend
    with the input loads' HWDGE ring."""
    from contextlib import ExitStack

    import concourse.tile as tile
    from concourse import bacc, mybir

    f32 = mybir.dt.float32
    bf16 = mybir.dt.bfloat16
    LR = mybir.ActivationFunctionType.Lrelu

    nc = bacc.Bacc("TRN2", target_bir_lowering=False)

    xt_d = nc.dram_tensor("xt", [F, E], bf16, kind="ExternalInput")
    # g.T duplicated: rows 0:64 and 64:128 both hold g.T (row packing)
    gt2_d = nc.dram_tensor("gt2", [128, E], bf16, kind="ExternalInput")
    w1a_d = nc.dram_tensor("w1a", [128, H], bf16, kind="ExternalInput")
    # W1[128:192] duplicated on both partition halves
    w1b2_d = nc.dram_tensor("w1b2", [128, H], bf16, kind="ExternalInput")
    # w2c[p, h*256+n] = W2[h*128+p, n]
    w2c_d = nc.dram_tensor("w2c", [128, 4 * 256], bf16, kind="ExternalInput")
    b1c_d = nc.dram_tensor("b1c", [128, 4], f32, kind="ExternalInput")
    b2c_d = nc.dram_tensor("b2c", [128, 2], f32, kind="ExternalInput")
    out_d = nc.dram_tensor("outT", [256, E], bf16, kind="ExternalOutput")

    with ExitStack() as ctx:
        tc = ctx.enter_context(tile.TileContext(nc))
        const = ctx.enter_context(tc.tile_pool(name="const", bufs=1))
        xpool = ctx.enter_context(tc.tile_pool(name="xp", bufs=3))
        gpool = ctx.enter_context(tc.tile_pool(name="gp", bufs=3))
        htps = ctx.enter_context(tc.tile_pool(name="htps", bufs=4, space="PSUM"))
        htsb = ctx.enter_context(tc.tile_pool(name="htsb", bufs=8))
        pjps = ctx.enter_context(tc.tile_pool(name="pjps", bufs=4, space="PSUM"))
        outp = ctx.enter_context(tc.tile_pool(name="outp", bufs=4))

        w1a_sb = const.tile_from(w1a_d[:])
        w1b2_sb = const.tile_from(w1b2_d[:])
        w2c_sb = const.tile_from(w2c_d[:])
        b1c_sb = const.tile_from(b1c_d[:])
        b2c_sb = const.tile_from(b2c_d[:])

        loop = None
        if reps > 1 and use_loop:
            loop = tc.For_i(
                0, reps, 1,
                hint_engines=(
                    mybir.EngineType.PE,
                    mybir.EngineType.Activation,
                    mybir.EngineType.DVE,
                    mybir.EngineType.SP,
                ),
            )
            loop.__enter__()

        for rep in range(1 if use_loop else reps):
            for ch in range(NCH):
                c0 = ch * LV
                xt_cur = xpool.tile([F, LV], bf16)
                nc.sync.dma_start(xt_cur[:], xt_d[:, c0 : c0 + LV])
                gt_cur = gpool.tile([128, LV], bf16)
                nc.sync.dma_start(gt_cur[:], gt2_d[:, c0 : c0 + LV])
                # output staging for this chunk: [128, LV] per branch half
                ost0 = outp.tile([128, LV], bf16, tag="ost", name=f"ost0_{rep}_{ch}")
                ost1 = outp.tile([128, LV], bf16, tag="ost", name=f"ost1_{rep}_{ch}")
                ost = [ost0, ost1]

                for blk in range(LV // EV):
                    off = blk * EV
                    bs = slice(off, off + EV)

                    hts = []
                    for ca in (0, 2):
                        cb = ca + 1
                        htp_a = htps.tile([128, EV], f32, tag="htp")
                        htp_b = htps.tile([128, EV], f32, tag="htp")
                        nc.tensor.matmul(
                            htp_a[:], w1a_sb[:, ca * 128 : (ca + 1) * 128],
                            xt_cur[:, bs], start=True, stop=False,
                        )
                        nc.tensor.matmul(
                            htp_b[:], w1a_sb[:, cb * 128 : (cb + 1) * 128],
                            xt_cur[:, bs], start=True, stop=False,
                        )
                        # row-packed K=64 pair: ca on rows 0:64, cb on 64:128
                        b_rows = slice(64, 128) if pack else slice(0, 64)
                        nc.tensor.matmul(
                            htp_a[:], w1b2_sb[0:64, ca * 128 : (ca + 1) * 128],
                            gt_cur[0:64, bs], start=False, stop=True,
                        )
                        nc.tensor.matmul(
                            htp_b[:], w1b2_sb[b_rows, cb * 128 : (cb + 1) * 128],
                            gt_cur[b_rows, bs], start=False, stop=True,
                        )
                        hsb_a = htsb.tile([128, EV], bf16, tag="hsb")
                        nc.scalar.activation(hsb_a[:], htp_a[:], LR,
                                             bias=b1c_sb[:, ca : ca + 1],
                                             scale=1.0, alpha=0.01)
                        hsb_b = htsb.tile([128, EV], bf16, tag="hsb")
                        nc.scalar.activation(hsb_b[:], htp_b[:], LR,
                                             bias=b1c_sb[:, cb : cb + 1],
                                             scale=1.0, alpha=0.01)
                        hts.extend([hsb_a, hsb_b])

                    for j in range(2):
                        pjp = pjps.tile([128, EV], f32)
                        for h in range(4):
                            wcol = h * 256 + j * 128
                            nc.tensor.matmul(
                                pjp[:], w2c_sb[:, wcol : wcol + 128],
                                hts[h][:], start=(h == 0), stop=(h == 3),
                            )
                        nc.vector.tensor_scalar_add(
                            ost[j][:, bs], pjp[:], b2c_sb[:, j : j + 1]
                        )

                store_eng = nc.gpsimd if store_gpsimd else nc.sync
                for j in range(2):
                    store_eng.dma_start(
                        out_d[j * 128 : (j + 1) * 128, c0 : c0 + LV], ost[j][:]
                    )

        if loop is not None:
            loop.__exit__(None, None, None)

    nc.compile()
    return nc


def _get_nc():
    global _NC
    if _NC is None:
        _NC = _build_nc()
    return _NC


def _prep_in_maps(x, g, W1, b1, W2, b2, idx):
    import ml_dtypes

    bf16 = ml_dtypes.bfloat16
    n_events = x.shape[0]

    # Gather (identity when parents_idxs == arange, the spec'd fill).
    if idx[0] == 0 and idx[-1] == n_events - 1 and np.array_equal(
        idx, np.arange(n_events, dtype=idx.dtype)
    ):
        xg, gg = x, g
    else:
        xg = x[idx]
        gg = g[idx % n_events]

    xT = np.ascontiguousarray(xg.T.astype(bf16))  # [F, E_TOT]
    gT = gg.T.astype(bf16)                        # [G, E_TOT]
    gT2 = np.ascontiguousarray(np.concatenate([gT, gT], axis=0))  # [128, E_TOT]
    w1a = np.ascontiguousarray(W1[:128, :].astype(bf16))
    w1b = W1[128:192, :].astype(bf16)
    w1b2 = np.ascontiguousarray(np.concatenate([w1b, w1b], axis=0))
    w2c = np.ascontiguousarray(
        W2.reshape(4, 128, 256).transpose(1, 0, 2).reshape(128, 1024).astype(bf16)
    )
    b1c = np.ascontiguousarray(b1.reshape(4, 128).T)
    b2c = np.ascontiguousarray(b2.reshape(2, 128).T)

    in_maps = []
    for c in range(N_CORES):
        sl = slice(c * E, (c + 1) * E)
        in_maps.append(
            {
                "xt": np.ascontiguousarray(xT[:, sl]),
                "gt2": np.ascontiguousarray(gT2[:, sl]),
                "w1a": w1a,
                "w1b2": w1b2,
                "w2c": w2c,
                "b1c": b1c,
                "b2c": b2c,
            }
        )
    return in_maps


def kernel(**inputs):
    global LAST_RESULTS, LAST_IN_MAPS
    from concourse.bass_utils import run_bass_kernel_spmd

    x = np.ascontiguousarray(np.asarray(inputs["x"], dtype=np.float32))
    g = np.ascontiguousarray(np.asarray(inputs["global_features"], dtype=np.float32))
    W1 = np.asarray(inputs["W1"], dtype=np.float32)
    b1 = np.asarray(inputs["b1"], dtype=np.float32)
    W2 = np.asarray(inputs["W2"], dtype=np.float32)
    b2 = np.asarray(inputs["b2"], dtype=np.float32)
    idx = np.asarray(inputs["parents_idxs"])

    n_events = x.shape[0]
    assert n_events == E_TOT and idx.shape[0] == E_TOT

    in_maps = _prep_in_maps(x, g, W1, b1, W2, b2, idx)
    LAST_IN_MAPS = in_maps

    nc = _get_nc()
    LAST_RESULTS = run_bass_kernel_spmd(
        nc, in_maps, core_ids=list(range(N_CORES)),
        trace=bool(int(os.environ.get("KERNEL_TRACE", "0"))),
    )

    out = np.empty((3 * E_TOT, F), dtype=np.float32)
    out[:E_TOT] = x
    for c in range(N_CORES):
        oT = LAST_RESULTS.results[c]["outT"]
        out[E_TOT + c * E : E_TOT + (c + 1) * E] = oT[:128].astype(np.float32).T
        out[2 * E_TOT + c * E : 2 * E_TOT + (c + 1) * E] = oT[128:].astype(np.float32).T
    return out
